# revision 1
# baseline (speedup 1.0000x reference)
"""Trainium2 Bass kernel for 3x3 valid conv (C_in=8, C_out=8, H=W=2048).

Strategy (spatial H-sharding across 8 cores):
  - Host splits x rows into 8 slabs of 256 output rows (+2 halo input rows)
    and packs each slab (fp16) into the exact SBUF layout the TensorE wants:
        xp[(ci, r), b, w] = slab[ci, h0(b) + r, w]
    for 19 row-blocks b (h0 = 14*b, last block 242), r = 0..15. Halo rows are
    duplicated host-side so every device load is a plain contiguous slice.
  - Three lhsT weight matrices (one per kw) of shape [K=128, M=112]:
        K = (ci, r), M = (co, j) with j = 0..13
        lhsT[kw][ci*16 + r, co*14 + j] = W[co, ci, r - j, kw]  (0 <= r-j <= 2)
  - Device per core: for each group of 4 blocks, one DMA loads
    [128, 4*2048] fp16; per block and per 512-wide column tile, 3
    accumulating matmuls (kw = 0,1,2; rhs shifted along the free axis)
    produce [112, 512] fp32 PSUM = out[(co, j), w]; PSUM tiles are copied
    (DVE/ACT alternating, fp32->fp16) into a [112, 4*2046] tile and stored
    with one DMA into op[(co, j), b, w]. Host scatters op back to
    (C, 2046, 2046) fp32.
"""

import numpy as np

import concourse.bass as bass
import concourse.mybir as mybir
import concourse.tile as tile
from concourse import bacc
from concourse.bass_utils import run_bass_kernel_spmd

# ---- problem geometry (hardcoded) ----
C = 8
H = 2048
W = 2048
KH = KW = 3
H_OUT = H - KH + 1   # 2046
W_OUT = W - KW + 1   # 2046
N_CORES = 8

ROWS_PER_CORE = 256          # output rows computed per core (core 7: 254 valid)
IN_ROWS = ROWS_PER_CORE + 2  # 258 input rows per core slab

J = 14                       # output rows per block
R = J + 2                    # 16 input rows per block
K = C * R                    # 128 contraction partitions
M = C * J                    # 112 output partitions
NBLK = 19                    # blocks per core
BLOCK_STARTS = [J * b for b in range(NBLK - 1)] + [ROWS_PER_CORE - J]
# h0(b) = 14*b for b<18, h0(18) = 242 (overlap-recompute tail)

COL_TILES = [(0, 512), (512, 512), (1024, 512), (1536, 510)]

IN_DT = mybir.dt.float16     # on-wire activation dtype
IN_NP = np.float16
OUT_DT = mybir.dt.float16    # on-wire output dtype (host upcasts)
OUT_NP = np.float16

GRP = 2                      # blocks per store DMA group
LOAD_GRP = 1                 # blocks per load DMA (divides into GRP groups)
Y_BUFS = 6
O_BUFS = 4


def build_nc(repeat: int = 1, mode: str = "full", grp: int = GRP,
             load_grp: int = LOAD_GRP, y_bufs: int = Y_BUFS, o_bufs: int = O_BUFS,
             load_eng: str = "pool"):
    do_mm = mode in ("full", "nocopy")
    do_copy = mode in ("full",)
    do_dma = mode in ("full", "nocopy", "dma")
    groups = [list(range(s, min(s + grp, NBLK))) for s in range(0, NBLK, grp)]
    nc = bacc.Bacc(
        "TRN2",
        target_bir_lowering=False,
        debug=False,
        num_devices=N_CORES,
    )
    xp = nc.dram_tensor("xp", [K, NBLK, W], IN_DT, kind="ExternalInput").ap()
    wts = nc.dram_tensor("wts", [KW, K, M], IN_DT, kind="ExternalInput").ap()
    op = nc.dram_tensor("op", [M, NBLK, W_OUT], OUT_DT, kind="ExternalOutput").ap()

    with tile.TileContext(nc) as tc:
        with (
            tc.tile_pool(name="wpool", bufs=1) as wpool,
            tc.tile_pool(name="ypool", bufs=y_bufs) as ypool,
            tc.tile_pool(name="opool", bufs=o_bufs) as opool,
            tc.tile_pool(name="pspool", bufs=8, space="PSUM") as pspool,
        ):
            wsb = wpool.tile([K, KW * M], IN_DT)
            for kw in range(KW):
                nc.sync.dma_start(wsb[:, kw * M:(kw + 1) * M], wts[kw])

            for rep_i in range(repeat):
                for blocks in groups:
                    g = len(blocks)
                    b0 = blocks[0]
                    # y is loaded in load_grp-block chunks for finer PE overlap
                    ys = []
                    for s in range(0, g, load_grp):
                        gl = min(load_grp, g - s)
                        yt = ypool.tile([K, gl * W], IN_DT, name="y", tag="y",
                                        padded_shape=[K, load_grp * W])
                        if do_dma:
                            if load_eng == "pool":
                                eng = nc.gpsimd
                            elif load_eng == "sp":
                                eng = nc.sync
                            else:  # alternate
                                eng = nc.gpsimd if (b0 + s) % (2 * load_grp) else nc.sync
                            eng.dma_start(yt[:], xp[:, b0 + s:b0 + s + gl, :])
                        ys.append(yt)

                    o = opool.tile([M, g * W_OUT], OUT_DT, name="o", tag="o",
                                   padded_shape=[M, grp * W_OUT])
                    for bi in range(g):
                        pss = []
                        for ti in range(len(COL_TILES)):
                            ps = pspool.tile([M, 512], mybir.dt.float32,
                                             name=f"ps{ti}", tag="ps")
                            pss.append(ps)
                        if do_mm:
                            y = ys[bi // load_grp]
                            yb = bi % load_grp
                            # kw-outer: consecutive MMs share the stationary side
                            for kw in range(KW):
                                for ti, (w0, n) in enumerate(COL_TILES):
                                    c0 = yb * W + w0 + kw
                                    nc.tensor.matmul(
                                        pss[ti][:, :n],
                                        lhsT=wsb[:, kw * M:(kw + 1) * M],
                                        rhs=y[:, c0:c0 + n],
                                        start=(kw == 0),
                                        stop=(kw == KW - 1),
                                    )
                        if do_copy:
                            for ti, (w0, n) in enumerate(COL_TILES):
                                dst = o[:, bi * W_OUT + w0:bi * W_OUT + w0 + n]
                                if ti % 2 == 0:
                                    nc.vector.tensor_copy(dst, pss[ti][:, :n])
                                else:
                                    nc.scalar.copy(dst, pss[ti][:, :n])
                    if not do_copy and do_dma:
                        # ablation modes: cheap writer so Tile allocates o
                        nc.vector.memset(o[:, :8], 0.0)
                    if do_dma:
                        nc.sync.dma_start(op[:, b0:b0 + g, :], o[:])

    nc.compile()
    return nc


def build_weight_lhst(weight: np.ndarray) -> np.ndarray:
    """weight: (C_out, C_in, 3, 3) fp32 -> (3, K, M) IN_NP."""
    wl = np.zeros((KW, K, M), np.float32)
    ci = np.arange(C)
    for kw in range(KW):
        for co in range(C):
            for j in range(J):
                for kh in range(KH):
                    r = j + kh
                    wl[kw, ci * R + r, co * J + j] = weight[co, :, kh, kw]
    return wl.astype(IN_NP)


def pack_core_input(slab: np.ndarray) -> np.ndarray:
    """slab: (C, IN_ROWS, W) fp16 -> xp (K, NBLK, W) fp16."""
    s0, s1, s2 = slab.strides
    # b = 0..17 uniform stride J; b = 18 special (h0 = 242)
    v = np.lib.stride_tricks.as_strided(
        slab, shape=(C, R, NBLK - 1, W), strides=(s0, s1, J * s1, s2)
    )
    xp = np.empty((C, R, NBLK, W), slab.dtype)
    xp[:, :, :NBLK - 1, :] = v
    xp[:, :, NBLK - 1, :] = slab[:, BLOCK_STARTS[-1]:BLOCK_STARTS[-1] + R, :]
    return xp.reshape(K, NBLK, W)


def unpack_core_output(op: np.ndarray) -> np.ndarray:
    """op: (M, NBLK, W_OUT) -> (C, ROWS_PER_CORE, W_OUT) float32."""
    op = op.reshape(C, J, NBLK, W_OUT)
    res = np.empty((C, ROWS_PER_CORE, W_OUT), np.float32)
    res[:, BLOCK_STARTS[-1]:, :] = op[:, :, NBLK - 1, :].astype(np.float32)
    res[:, :J * (NBLK - 1), :] = (
        op[:, :, :NBLK - 1, :].transpose(0, 2, 1, 3).reshape(C, J * (NBLK - 1), W_OUT)
    )
    return res


def shard_inputs(x: np.ndarray, weight: np.ndarray):
    xc = np.ascontiguousarray(x).astype(IN_NP)
    wl = build_weight_lhst(weight)
    in_maps = []
    for i in range(N_CORES):
        lo = i * ROWS_PER_CORE
        hi = min(lo + IN_ROWS, H)
        if hi - lo == IN_ROWS:
            slab = xc[:, lo:hi, :]
        else:
            slab = np.zeros((C, IN_ROWS, W), IN_NP)
            slab[:, :hi - lo, :] = xc[:, lo:hi, :]
        in_maps.append({"xp": pack_core_input(slab), "wts": wl})
    return in_maps


def unshard_output(results) -> np.ndarray:
    parts = []
    for i in range(N_CORES):
        rows = ROWS_PER_CORE if i < N_CORES - 1 else H_OUT - (N_CORES - 1) * ROWS_PER_CORE
        parts.append(unpack_core_output(results[i]["op"])[:, :rows, :])
    return np.concatenate(parts, axis=1)


_NC_CACHE = None


def _get_nc():
    global _NC_CACHE
    if _NC_CACHE is None:
        _NC_CACHE = build_nc()
    return _NC_CACHE


def run(inputs: dict, **spmd_kwargs):
    """Run the conv on 8 NeuronCores. Returns (full_output, BassKernelResults)."""
    in_maps = shard_inputs(np.asarray(inputs["x"]), np.asarray(inputs["weight"]))
    nc = _get_nc()
    res = run_bass_kernel_spmd(nc, in_maps, core_ids=list(range(N_CORES)), **spmd_kwargs)
    return unshard_output(res.results).astype(np.float32), res


def kernel(**inputs) -> np.ndarray:
    out, _ = run(inputs)
    return out



# revision 18
# speedup vs baseline: 1.0503x; 1.0503x over previous
"""Trainium2 Bass kernel for 3x3 valid conv (C_in=8, C_out=8, H=W=2048).

Strategy (spatial H-sharding across 8 cores):
  - Host splits x rows into 8 slabs of 256 output rows (+2 halo input rows)
    and packs each slab (fp16) into the SBUF layout the TensorE wants:
        xp[(ci, r), b, w] = slab[ci, 14*b + r, w]
    for 18 full row-blocks b, r = 0..15 (J=14 output rows per block), plus a
    small tail block (J=4): xt[(ci, r6)] = slab[ci, 252 + r6], r6 = 0..5.
    Halo rows between blocks are duplicated host-side so every device load is
    a plain contiguous slice.
  - lhsT weights packed in one [128, 384] fp16 tensor:
        cols kw*112 + co*14 + j   (kw=0..2): full-block lhsT
            wt[ci*16 + r, kw*112 + co*14 + j] = W[co, ci, r - j, kw]
        cols 336 + kw*32 + co*4 + j: tail lhsT on partitions ci*6 + r6.
  - Device per core, per block: per 512-wide column tile, 3 accumulating
    matmuls (kw = 0,1,2; rhs shifted along the free axis) produce
    [112, 512] fp32 PSUM = out[(co, j), w]; PSUM tiles are copied
    (DVE/ACT alternating, fp32->fp16) into [112, 2046] tiles and stored
    per block. Schedule details for latency:
      * a few K=1 dummy matmuls at t~0 keep the PE p-state ramp warming
        while the first input block is still in flight,
      * block 0 is loaded in two pieces (first column tile via SP/HWDGE,
        rest via Pool/SWDGE) so the first real matmul starts ~3us in,
      * the last full block and the tail store per-column-tile so the
        final DMAs are small.
  - Host scatters op [112, 18, 2046] + ot [32, 2046] back to (C, rows, 2046).
"""

import numpy as np

import concourse.bass as bass
import concourse.mybir as mybir
import concourse.tile as tile
from concourse import bacc
from concourse.bass_utils import run_bass_kernel_spmd

# ---- problem geometry (hardcoded) ----
C = 8
H = 2048
W = 2048
KH = KW = 3
H_OUT = H - KH + 1   # 2046
W_OUT = W - KW + 1   # 2046
N_CORES = 8

ROWS_PER_CORE = 256          # output rows computed per core (core 7: 254 valid)
IN_ROWS = ROWS_PER_CORE + 2  # 258 input rows per core slab

J = 14                       # output rows per full block
R = J + 2                    # 16 input rows per full block
K = C * R                    # 128 contraction partitions
M = C * J                    # 112 output partitions
NBLK = 18                    # full blocks per core (covers rows 0..251)
JT = 4                       # tail block output rows (252..255)
RT = JT + 2                  # 6 tail input rows (252..257)
KT = C * RT                  # 48 tail contraction partitions
MT = C * JT                  # 32 tail output partitions
TAIL_START = NBLK * J        # 252

COL_TILES = [(0, 512), (512, 512), (1024, 512), (1536, 510)]
# tail column tiles: finer near the end so the final copy+store chain is tiny
TAIL_TILES = [(0, 512), (512, 512), (1024, 512), (1536, 384), (1920, 126)]

IN_DT = mybir.dt.float16     # on-wire activation dtype
IN_NP = np.float16
OUT_DT = mybir.dt.float16    # on-wire output dtype (host upcasts)
OUT_NP = np.float16

WCOLS = KW * M + KW * MT     # 336 + 96 = 432 weight columns
Y_BUFS = 6
O_BUFS = 5
N_DUMMY = 5                  # PE warm-up matmuls
PIECE_A = 516                # first-piece width of block 0 (covers tile 0)


def build_nc(repeat: int = 1, mode: str = "full", y_bufs: int = Y_BUFS,
             o_bufs: int = O_BUFS, n_dummy: int = N_DUMMY,
             piece_a: int = PIECE_A):
    do_mm = mode in ("full", "nocopy")
    do_copy = mode in ("full",)
    do_dma = mode in ("full", "nocopy", "dma")
    nc = bacc.Bacc(
        "TRN2",
        target_bir_lowering=False,
        debug=False,
        num_devices=N_CORES,
    )
    xp = nc.dram_tensor("xp", [K, NBLK, W], IN_DT, kind="ExternalInput").ap()
    xt = nc.dram_tensor("xt", [KT, W], IN_DT, kind="ExternalInput").ap()
    wt = nc.dram_tensor("wt", [K, WCOLS], IN_DT, kind="ExternalInput").ap()
    op = nc.dram_tensor("op", [M, NBLK, W_OUT], OUT_DT, kind="ExternalOutput").ap()
    ot = nc.dram_tensor("ot", [MT, W_OUT], OUT_DT, kind="ExternalOutput").ap()

    f32 = mybir.dt.float32

    with tile.TileContext(nc) as tc:
        with (
            tc.tile_pool(name="wpool", bufs=1) as wpool,
            tc.tile_pool(name="dpool", bufs=1) as dpool,
            tc.tile_pool(name="ypool", bufs=y_bufs) as ypool,
            tc.tile_pool(name="ytpool", bufs=1) as ytpool,
            tc.tile_pool(name="opool", bufs=o_bufs) as opool,
            tc.tile_pool(name="otpool", bufs=4) as otpool,
            tc.tile_pool(name="pspool", bufs=8, space="PSUM") as pspool,
        ):
            wsb = wpool.tile([K, WCOLS], IN_DT)
            dmy = dpool.tile([1, 512], IN_DT)

            # PE warm-up: memset a tiny tile, then K=1 dummy matmuls chained
            # on one PSUM tile; they occupy the PE (starting its p-state
            # ramp) while the first real input lands.
            if n_dummy and do_mm:
                nc.vector.memset(dmy[:], 0.0)
                dps = pspool.tile([1, 512], f32, name="dps", tag="ps")
                for _ in range(n_dummy):
                    nc.tensor.matmul(dps[:, :], lhsT=dmy[0:1, 0:1],
                                     rhs=dmy[0:1, :], start=True, stop=True)

            for rep_i in range(repeat):
                # Block 0 splits into two SEPARATE tiles (A: tiles 0-1 via
                # SP/HWDGE, B: tiles 2-3 via SP/HWDGE second) so the first
                # matmul depends only on A + weights; weights ride Pool's
                # first SWDGE slot. The tail load xt goes late on SP.
                yA = ypool.tile([K, piece_a], IN_DT, name="yA", tag="y",
                                padded_shape=[K, W])
                yB = ypool.tile([K, W - 512], IN_DT, name="yB", tag="y",
                                padded_shape=[K, W])
                if do_dma:
                    nc.sync.dma_start(yA[:], xp[:, 0, 0:piece_a])
                    if rep_i == 0:
                        nc.gpsimd.dma_start(wsb[:], wt)
                    nc.sync.dma_start(yB[:], xp[:, 0, 512:W])
                # tail load slots into the Pool queue mid-stream (after b8's
                # load) so it neither delays the startup DMAs nor arrives late
                yt = ytpool.tile([KT, W], IN_DT)
                ys = [None]
                for b in range(1, NBLK):
                    yb = ypool.tile([K, W], IN_DT, name="y", tag="y",
                                    padded_shape=[K, W])
                    if do_dma:
                        nc.gpsimd.dma_start(yb[:], xp[:, b, :])
                        if b == 8:
                            with tc.tile_wait_until(0.008):
                                nc.gpsimd.dma_start(yt[:], xt)
                    ys.append(yb)

                for b in range(NBLK):
                    o = opool.tile([M, W_OUT], OUT_DT, name="o", tag="o",
                                   padded_shape=[M, W_OUT])
                    # ti-outer everywhere: each column tile finishes its 3-kw
                    # accumulation back-to-back, so its PSUM copy (and the
                    # store gen behind it) chases the PE with ~1-tile lag.
                    for ti, (w0, n) in enumerate(COL_TILES):
                        ps = pspool.tile([M, 512], f32, name=f"ps{ti}", tag="ps")
                        if do_mm:
                            for kw in range(KW):
                                if b == 0:
                                    rhs = (yA[:, w0 + kw:w0 + kw + n] if ti < 1
                                           else yB[:, w0 - 512 + kw:
                                                   w0 - 512 + kw + n])
                                else:
                                    rhs = ys[b][:, w0 + kw:w0 + kw + n]
                                nc.tensor.matmul(
                                    ps[:, :n],
                                    lhsT=wsb[:, kw * M:(kw + 1) * M],
                                    rhs=rhs,
                                    start=(kw == 0),
                                    stop=(kw == KW - 1),
                                )
                        if do_copy:
                            dst = o[:, w0:w0 + n]
                            if ti % 2 == 0:
                                nc.vector.tensor_copy(dst, ps[:, :n])
                            else:
                                nc.scalar.copy(dst, ps[:, :n])
                    if not do_copy and do_dma:
                        nc.vector.memset(o[:, :8], 0.0)
                    if do_dma:
                        if b == NBLK - 1:
                            # last full block: two half stores so the first
                            # half's DMA drains during the tail's compute
                            nc.sync.dma_start(op[:, b, 0:1024], o[:, 0:1024])
                            nc.sync.dma_start(op[:, b, 1024:W_OUT],
                                              o[:, 1024:W_OUT])
                        else:
                            nc.sync.dma_start(op[:, b, :], o[:])

                # tail block (J=4): ti-outer, per-tile copies into ONE SBUF
                # tile, single small store at the very end (one HWDGE gen on
                # the final chain); final tile is tiny (126 cols).
                ob = otpool.tile([MT, W_OUT], OUT_DT, name="otb", tag="ot")
                for ti, (w0, n) in enumerate(TAIL_TILES):
                    ps = pspool.tile([MT, 512], f32, name=f"tps{ti}", tag="ps")
                    if do_mm:
                        for kw in range(KW):
                            nc.tensor.matmul(
                                ps[:, :n],
                                lhsT=wsb[0:KT, KW * M + kw * MT:
                                         KW * M + (kw + 1) * MT],
                                rhs=yt[:, w0 + kw:w0 + kw + n],
                                start=(kw == 0),
                                stop=(kw == KW - 1),
                            )
                    if do_copy:
                        if ti % 2 == 0:
                            nc.vector.tensor_copy(ob[:, w0:w0 + n], ps[:, :n])
                        else:
                            nc.scalar.copy(ob[:, w0:w0 + n], ps[:, :n])
                if not do_copy and do_dma:
                    nc.vector.memset(ob[:, :8], 0.0)
                if do_dma:
                    nc.sync.dma_start(ot[:, 0:1024], ob[:, 0:1024])
                    nc.sync.dma_start(ot[:, 1024:W_OUT], ob[:, 1024:W_OUT])

    nc.compile()
    return nc


def build_weights(weight: np.ndarray) -> np.ndarray:
    """weight: (C_out, C_in, 3, 3) fp32 -> (K, WCOLS) IN_NP merged lhsT."""
    wl = np.zeros((K, WCOLS), np.float32)
    for kw in range(KW):
        for co in range(C):
            for j in range(J):
                for kh in range(KH):
                    r = j + kh
                    wl[np.arange(C) * R + r, kw * M + co * J + j] = \
                        weight[co, :, kh, kw]
            for j in range(JT):
                for kh in range(KH):
                    r = j + kh
                    wl[np.arange(C) * RT + r, KW * M + kw * MT + co * JT + j] = \
                        weight[co, :, kh, kw]
    return wl.astype(IN_NP)


def pack_core_input(slab: np.ndarray):
    """slab: (C, IN_ROWS, W) fp16 -> (xp [K, NBLK, W], xt [KT, W]) fp16."""
    s0, s1, s2 = slab.strides
    v = np.lib.stride_tricks.as_strided(
        slab, shape=(C, R, NBLK, W), strides=(s0, s1, J * s1, s2)
    )
    xp = np.ascontiguousarray(v).reshape(K, NBLK, W)
    xt = np.ascontiguousarray(
        slab[:, TAIL_START:TAIL_START + RT, :]).reshape(KT, W)
    return xp, xt


def unpack_core_output(op: np.ndarray, ot: np.ndarray) -> np.ndarray:
    """op: (M, NBLK, W_OUT), ot: (MT, W_OUT) -> (C, ROWS_PER_CORE, W_OUT)."""
    res = np.empty((C, ROWS_PER_CORE, W_OUT), np.float32)
    res[:, :NBLK * J, :] = (
        op.reshape(C, J, NBLK, W_OUT).transpose(0, 2, 1, 3)
        .reshape(C, NBLK * J, W_OUT).astype(np.float32)
    )
    res[:, TAIL_START:, :] = ot.reshape(C, JT, W_OUT).astype(np.float32)
    return res


def shard_inputs(x: np.ndarray, weight: np.ndarray):
    xc = np.ascontiguousarray(x).astype(IN_NP)
    wl = build_weights(weight)
    in_maps = []
    for i in range(N_CORES):
        lo = i * ROWS_PER_CORE
        hi = min(lo + IN_ROWS, H)
        if hi - lo == IN_ROWS:
            slab = xc[:, lo:hi, :]
        else:
            slab = np.zeros((C, IN_ROWS, W), IN_NP)
            slab[:, :hi - lo, :] = xc[:, lo:hi, :]
        xp, xt = pack_core_input(slab)
        in_maps.append({"xp": xp, "xt": xt, "wt": wl})
    return in_maps


def unshard_output(results) -> np.ndarray:
    parts = []
    for i in range(N_CORES):
        rows = ROWS_PER_CORE if i < N_CORES - 1 else H_OUT - (N_CORES - 1) * ROWS_PER_CORE
        full = unpack_core_output(results[i]["op"], results[i]["ot"])
        parts.append(full[:, :rows, :])
    return np.concatenate(parts, axis=1)


_NC_CACHE = None


def _get_nc():
    global _NC_CACHE
    if _NC_CACHE is None:
        _NC_CACHE = build_nc()
    return _NC_CACHE


def run(inputs: dict, **spmd_kwargs):
    """Run the conv on 8 NeuronCores. Returns (full_output, BassKernelResults)."""
    in_maps = shard_inputs(np.asarray(inputs["x"]), np.asarray(inputs["weight"]))
    nc = _get_nc()
    res = run_bass_kernel_spmd(nc, in_maps, core_ids=list(range(N_CORES)), **spmd_kwargs)
    return unshard_output(res.results).astype(np.float32), res


def kernel(**inputs) -> np.ndarray:
    out, _ = run(inputs)
    return out
